# revision 1
# baseline (speedup 1.0000x reference)
"""Expert-parallel MoE MLP kernel for Trainium2 (8 NeuronCores, 1 expert/core).

Problem: inputs [1, 8, 16384, 512], per-expert 2-layer GELU MLP
  h   = gelu(x @ W1[e] + b1[e])      # [16384, 2048]
  out = h @ W2[e] + b2[e]            # [16384, 512]

Per-core dataflow (activations kept transposed, d-on-partitions):
  1. DMA x block [512t, 512d] -> SBUF natural layout, PE-transpose -> xT [d, t]
     (transposes run one block ahead of the matmul pipeline)
  2. L1: psum[f,t] = sum_k matmul(lhsT=W1[dk, f], rhs=xT[dk, t])   (fp32r)
  3. ScalarE Gelu(+b1 per-partition bias) psum -> hT sbuf [f, t]
  4. L2: psum[t,d'] = sum_k matmul(lhsT=hT[fk, t], rhs=W2[fk, d']) (fp32r)
     -> output lands in natural token-major layout, no output transpose
  5. DVE add b2 (broadcast) psum -> sbuf, DMA out.
"""

import os
import numpy as np

E, C, D, F = 8, 16384, 512, 2048
P = 128
TBLK = 512  # tokens per block
MM_DT = "float32r"  # PE 1 cyc/row at N>=256 (vs 4 for float32)

_CACHE = {}


def _build(T, act="Gelu_apprx_tanh"):
    import concourse.mybir as mybir
    import concourse.tile as tile
    from concourse import bacc
    from concourse.masks import make_identity

    f32 = mybir.dt.float32
    mm_dt = getattr(mybir.dt, MM_DT)
    gelu_fn = getattr(mybir.ActivationFunctionType, act)

    nc = bacc.Bacc("TRN2", target_bir_lowering=False, debug=False)

    x_d = nc.dram_tensor("x", [T, D], f32, kind="ExternalInput").ap()
    w1_d = nc.dram_tensor("w1", [D, F], f32, kind="ExternalInput").ap()
    b1_d = nc.dram_tensor("b1", [F], f32, kind="ExternalInput").ap()
    w2_d = nc.dram_tensor("w2", [F, D], f32, kind="ExternalInput").ap()
    b2_d = nc.dram_tensor("b2", [D], f32, kind="ExternalInput").ap()
    o_d = nc.dram_tensor("out", [T, D], f32, kind="ExternalOutput").ap()

    KD = D // P   # 4  k-tiles (d) for layer 1
    KF = F // P   # 16 k-tiles (f) for layer 2
    NB = T // TBLK
    JT = TBLK // P  # 4 token sub-tiles per block

    with tile.TileContext(nc) as tc:
        with (
            tc.tile_pool(name="consts", bufs=1) as consts,
            tc.tile_pool(name="xn", bufs=2) as xn_pool,
            tc.tile_pool(name="xt", bufs=2) as xt_pool,
            tc.tile_pool(name="ht", bufs=1) as ht_pool,
            tc.tile_pool(name="ot", bufs=2) as ot_pool,
            tc.tile_pool(name="pxt", bufs=3, space="PSUM") as pxt_pool,
            tc.tile_pool(name="ph", bufs=3, space="PSUM") as ph_pool,
            tc.tile_pool(name="po", bufs=2, space="PSUM") as po_pool,
        ):
            # --- setup: DMA order tracks the startup critical path ---
            ident = consts.tile([P, P], f32)
            make_identity(nc, ident[:])

            def load_block(blk):
                xn = xn_pool.tile([P, JT, D], f32, name="xn", tag="xn")
                t0 = blk * TBLK
                nc.sync.dma_start(
                    xn[:], x_d[t0 : t0 + TBLK, :].rearrange("(j p) d -> p j d", p=P)
                )
                return xn

            xns = {0: load_block(0)}
            if NB > 1:
                xns[1] = load_block(1)

            w1_sb = consts.tile([P, KD, F], mm_dt)
            w1_r = w1_d.rearrange("(k p) f -> p k f", p=P).bitcast(mm_dt)
            for k in range(KD):
                nc.sync.dma_start(w1_sb[:, k, :], w1_r[:, k, :])
            b1_sb = consts.tile([P, KF], f32)
            nc.sync.dma_start(b1_sb[:], b1_d.rearrange("(k p) -> p k", p=P))

            w2_sb = consts.tile([P, KF, D], mm_dt)
            w2_r = w2_d.rearrange("(k p) d -> p k d", p=P).bitcast(mm_dt)
            for k in range(KF):
                nc.sync.dma_start(w2_sb[:, k, :], w2_r[:, k, :])
            b2_bc = consts.tile([P, D], f32)
            nc.sync.dma_start(b2_bc[:], b2_d.unsqueeze(0).partition_broadcast(P))

            def transp(xn):
                """PE-transpose one x block -> per-k xT tiles [d_p, t]."""
                xts = []
                for k in range(KD):
                    pxt = pxt_pool.tile([P, TBLK], f32)
                    for j in range(JT):
                        nc.tensor.transpose(
                            pxt[:, j * P : (j + 1) * P],
                            xn[:, j, k * P : (k + 1) * P],
                            ident[:],
                        )
                    xt_k = xt_pool.tile(
                        [P, TBLK], mm_dt, name=f"xt{k}", tag=f"xt{k}"
                    )
                    nc.vector.tensor_copy(xt_k[:], pxt[:])
                    xts.append(xt_k)
                return xts

            def layer1(xts):
                hts = []
                for f in range(KF):
                    ph = ph_pool.tile([P, TBLK], f32)
                    for k in range(KD):
                        nc.tensor.matmul(
                            ph[:],
                            w1_sb[:, k, f * P : (f + 1) * P],
                            xts[k][:],
                            start=(k == 0),
                            stop=(k == KD - 1),
                        )
                    ht_f = ht_pool.tile(
                        [P, TBLK], mm_dt, name=f"ht{f}", tag=f"ht{f}"
                    )
                    nc.scalar.activation(
                        ht_f[:], ph[:], gelu_fn, bias=b1_sb[:, f : f + 1]
                    )
                    hts.append(ht_f)
                return hts

            def layer2(blk, hts):
                t0 = blk * TBLK
                for j in range(JT):
                    po = po_pool.tile([P, D], f32)
                    for k in range(KF):
                        nc.tensor.matmul(
                            po[:],
                            hts[k][:, j * P : (j + 1) * P],
                            w2_sb[:, k, :],
                            start=(k == 0),
                            stop=(k == KF - 1),
                        )
                    ot_j = ot_pool.tile([P, D], f32, name=f"ot{j}", tag=f"ot{j}")
                    nc.vector.tensor_add(ot_j[:], po[:], b2_bc[:])
                    nc.sync.dma_start(
                        o_d[t0 + j * P : t0 + (j + 1) * P, :], ot_j[:]
                    )

            # PE order: T0 | T1 L1(0) L2(0) | T2 L1(1) L2(1) | ...
            # transposes run one block ahead of the L1/L2 pipeline
            xts_cur = transp(xns.pop(0))
            for blk in range(NB):
                xts_next = None
                if blk + 1 < NB:
                    xts_next = transp(xns.pop(blk + 1))
                if blk + 2 < NB:
                    xns[blk + 2] = load_block(blk + 2)
                hts = layer1(xts_cur)
                layer2(blk, hts)
                if xts_next is not None:
                    xts_cur = xts_next

    nc.compile()
    return nc


def _get_nc(T):
    if T not in _CACHE:
        _CACHE[T] = _build(T)
    return _CACHE[T]


def kernel(inputs, W1, b1, W2, b2):
    from concourse.bass_utils import run_bass_kernel_spmd

    inputs = np.ascontiguousarray(np.asarray(inputs, dtype=np.float32))
    W1 = np.ascontiguousarray(np.asarray(W1, dtype=np.float32))
    b1 = np.ascontiguousarray(np.asarray(b1, dtype=np.float32))
    W2 = np.ascontiguousarray(np.asarray(W2, dtype=np.float32))
    b2 = np.ascontiguousarray(np.asarray(b2, dtype=np.float32))

    nc = _get_nc(C)
    in_maps = [
        {
            "x": inputs[0, e],
            "w1": W1[e],
            "b1": b1[e],
            "w2": W2[e],
            "b2": b2[e],
        }
        for e in range(E)
    ]
    trace = os.environ.get("KERNEL_TRACE", "0") == "1"
    res = run_bass_kernel_spmd(
        nc, in_maps, core_ids=list(range(E)), trace=trace
    )
    if trace:
        kernel.last_exec_time_ns = res.exec_time_ns
    out = np.stack([res.results[e]["out"] for e in range(E)], axis=0)[None]
    return out



# revision 2
# speedup vs baseline: 1.0776x; 1.0776x over previous
"""Expert-parallel MoE MLP kernel for Trainium2 (8 NeuronCores, 1 expert/core).

Problem: inputs [1, 8, 16384, 512], per-expert 2-layer GELU MLP
  h   = gelu(x @ W1[e] + b1[e])      # [16384, 2048]
  out = h @ W2[e] + b2[e]            # [16384, 512]

v2 dataflow (all matmul operands bf16; PE runs ONLY the 128 N=512 matmuls
per 512-token block -- x transposes moved to the DMA xbar):
  1. Host casts x/W1/W2 to bf16 (rel err ~2e-3, gate is 2e-2).
  2. xT tiles [d_p, t] land in SBUF directly via dma_start(transpose=True)
     (16x128 xbar tiles, ~14ns each, fully off the PE).
  3. L1: psum[f,t] = sum_k matmul(lhsT=W1[dk, f], rhs=xT[dk, t])  (bf16+FWL)
  4. ScalarE Gelu(+b1 per-partition bias) psum -> hT sbuf [f, t] bf16
  5. L2: psum[t,d] = sum_k matmul(lhsT=hT[fk, t], rhs=W2[fk, d])  (bf16+FWL)
     -> natural token-major layout, no output transpose
  6. DVE adds b2 (broadcast) psum -> f32 sbuf, DMA out.
Startup: W1/b1/W2/b2 on the Activation hwdge queue, xT loads on SP queue,
so the first matmul can start ~6us in.
"""

import os
import numpy as np

E, C, D, F = 8, 16384, 512, 2048
P = 128
TBLK = 512  # tokens per block
KD = D // P   # 4  k-tiles (d) for layer 1
KF = F // P   # 16 k-tiles (f) for layer 2
JT = TBLK // P  # 4 token sub-tiles per block

_CACHE = {}


def _build(T, act="Gelu_apprx_tanh"):
    import concourse.mybir as mybir
    import concourse.tile as tile
    from concourse import bacc

    f32 = mybir.dt.float32
    bf16 = mybir.dt.bfloat16
    gelu_fn = getattr(mybir.ActivationFunctionType, act)

    nc = bacc.Bacc("TRN2", target_bir_lowering=False, debug=False)

    x_d = nc.dram_tensor("x", [T, D], bf16, kind="ExternalInput").ap()
    w1_d = nc.dram_tensor("w1", [D, F], bf16, kind="ExternalInput").ap()
    b1_d = nc.dram_tensor("b1", [F], f32, kind="ExternalInput").ap()
    w2_d = nc.dram_tensor("w2", [F, D], bf16, kind="ExternalInput").ap()
    b2_d = nc.dram_tensor("b2", [D], f32, kind="ExternalInput").ap()
    o_d = nc.dram_tensor("out", [T, D], f32, kind="ExternalOutput").ap()

    NB = T // TBLK

    with tile.TileContext(nc) as tc:
        with (
            tc.tile_pool(name="consts", bufs=1) as consts,
            tc.tile_pool(name="xt", bufs=3) as xt_pool,
            tc.tile_pool(name="ht", bufs=1) as ht_pool,
            tc.tile_pool(name="ot", bufs=2) as ot_pool,
            tc.tile_pool(name="ph", bufs=3, space="PSUM") as ph_pool,
            tc.tile_pool(name="po", bufs=2, space="PSUM") as po_pool,
        ):
            # --- setup: weights/biases on the Activation hwdge queue,
            # xT tiles on the SP queue, in parallel ---
            w1_sb = consts.tile([P, KD, F], bf16)
            w1_r = w1_d.rearrange("(k p) f -> p k f", p=P)
            for k in range(KD):
                nc.scalar.dma_start(w1_sb[:, k, :], w1_r[:, k, :])
            b1_sb = consts.tile([P, KF], f32)
            nc.scalar.dma_start(b1_sb[:], b1_d.rearrange("(k p) -> p k", p=P))

            def load_xt(blk):
                """xbar-transposed DMA: x[t0:t0+TBLK, kP:(k+1)P] -> [d_p, t]."""
                t0 = blk * TBLK
                xts = []
                for k in range(KD):
                    xt_k = xt_pool.tile(
                        [P, TBLK], bf16, name=f"xt{k}", tag=f"xt{k}"
                    )
                    nc.sync.dma_start(
                        xt_k[:],
                        x_d[t0 : t0 + TBLK, k * P : (k + 1) * P],
                        transpose=True,
                    )
                    xts.append(xt_k)
                return xts

            xts = {b: load_xt(b) for b in range(min(3, NB))}

            w2_sb = consts.tile([P, KF, D], bf16)
            w2_r = w2_d.rearrange("(k p) d -> p k d", p=P)
            for k in range(KF):
                nc.scalar.dma_start(w2_sb[:, k, :], w2_r[:, k, :])
            b2_bc = consts.tile([P, D], f32)
            nc.scalar.dma_start(b2_bc[:], b2_d.unsqueeze(0).partition_broadcast(P))

            def layer1(xts_cur):
                hts = []
                for f in range(KF):
                    ph = ph_pool.tile([P, TBLK], f32)
                    for k in range(KD):
                        nc.tensor.matmul(
                            ph[:],
                            w1_sb[:, k, f * P : (f + 1) * P],
                            xts_cur[k][:],
                            start=(k == 0),
                            stop=(k == KD - 1),
                        )
                    ht_f = ht_pool.tile(
                        [P, TBLK], bf16, name=f"ht{f}", tag=f"ht{f}"
                    )
                    nc.scalar.activation(
                        ht_f[:], ph[:], gelu_fn, bias=b1_sb[:, f : f + 1]
                    )
                    hts.append(ht_f)
                return hts

            def layer2(blk, hts):
                t0 = blk * TBLK
                for j in range(JT):
                    po = po_pool.tile([P, D], f32)
                    for k in range(KF):
                        nc.tensor.matmul(
                            po[:],
                            hts[k][:, j * P : (j + 1) * P],
                            w2_sb[:, k, :],
                            start=(k == 0),
                            stop=(k == KF - 1),
                        )
                    ot_j = ot_pool.tile([P, D], f32, name=f"ot{j}", tag=f"ot{j}")
                    nc.vector.tensor_add(ot_j[:], po[:], b2_bc[:])
                    nc.sync.dma_start(
                        o_d[t0 + j * P : t0 + (j + 1) * P, :], ot_j[:]
                    )

            for blk in range(NB):
                if blk + 3 < NB:
                    xts[blk + 3] = load_xt(blk + 3)
                hts = layer1(xts.pop(blk))
                layer2(blk, hts)

    nc.compile()
    return nc


def _get_nc(T):
    if T not in _CACHE:
        _CACHE[T] = _build(T)
    return _CACHE[T]


def kernel(inputs, W1, b1, W2, b2):
    import ml_dtypes
    from concourse.bass_utils import run_bass_kernel_spmd

    bf16 = ml_dtypes.bfloat16
    inputs = np.asarray(inputs, dtype=np.float32).astype(bf16)
    W1 = np.asarray(W1, dtype=np.float32).astype(bf16)
    b1 = np.ascontiguousarray(np.asarray(b1, dtype=np.float32))
    W2 = np.asarray(W2, dtype=np.float32).astype(bf16)
    b2 = np.ascontiguousarray(np.asarray(b2, dtype=np.float32))

    nc = _get_nc(C)
    in_maps = [
        {
            "x": np.ascontiguousarray(inputs[0, e]),
            "w1": np.ascontiguousarray(W1[e]),
            "b1": b1[e],
            "w2": np.ascontiguousarray(W2[e]),
            "b2": b2[e],
        }
        for e in range(E)
    ]
    trace = os.environ.get("KERNEL_TRACE", "0") == "1"
    res = run_bass_kernel_spmd(
        nc, in_maps, core_ids=list(range(E)), trace=trace
    )
    if trace:
        kernel.last_exec_time_ns = res.exec_time_ns
    out = np.stack([res.results[e]["out"] for e in range(E)], axis=0)[None]
    return out
